# revision 5
# baseline (speedup 1.0000x reference)
"""Trainium2 Bass kernel for nn_AttLayer (4-head attention, softmax over queries).

Sharding: data-parallel over batch. 8 batch elements -> 8 NeuronCores, zero
collectives.

Key algebraic restructuring: with C=64 channels the attention is rank-65.
Folding the projections through the score/value contractions (bias rows
appended via the augmented-ones trick):

  R_h       = G_h^T-contracted input               G_h = Wk_aug_h @ Wq_aug_h^T
  scoresT_h = Xa^T R_h                             (= Xa^T G_h Xa, 65 x 65 G)
  es        = exp(SCALE * scoresT)                 row-sum den fused into the
                                                   exp activation (accum_out)
  xtr[j,c]  = XaT[j,c] / den[j]                    reciprocal folded into the
                                                   65-wide transposed input
  M2_h[c,i] = sum_j xtr[j,c] * es[j,i]             (65 x 1024)
  out2     += F_h^T @ M2_h                         F_h = Wv_aug_h @ Wout_h
  out       = sum_h out2_h + (x + b_out)           residual+bias pre-summed
                                                   into the accumulator

G_h and F_h are computed on the host in f32 (exact). Everything on-chip is
bf16 matmuls with f32 PSUM accumulation; the exp/normalize core is the
critical path (ScalarEngine). Pipelining:
  - dummy warm-up matmuls on a zeroed scratch tile run during the input DMA
    so the PE HAM clock-gate is released before the first real matmul
  - M2 for head h runs at lag-1 (during steps jt+1 of the same head), its
    tail (jt=7 M2, PSUM->SBUF conv, out2, accumulate) spreads over the first
    steps of head h+1, one i-chunk per chain step
"""

import numpy as np
import ml_dtypes

import concourse.tile as tile
from concourse import bacc, mybir
from concourse.bass_utils import run_bass_kernel_spmd

NH = 4          # heads
D = 640         # per-head dim
C = 64          # channels
CA = C + 1      # augmented (ones row)
SEQ = 1024      # 32*32
SCALE = float(D) ** -0.5
N_CORES = 8
FP = mybir.dt.float32
BF = mybir.dt.bfloat16

JT = SEQ // 128     # 8 j-tiles (128 keys each)
IC = SEQ // 512     # 2 i-chunks (512 queries each)
N_WARM = 12         # HAM warm-up matmuls during the input DMA

AF = mybir.ActivationFunctionType
ALU = mybir.AluOpType

# packed qa blob columns: xa | gt | ff | b_out (f32 bit-pattern in 2 bf16 cols)
W_QA = SEQ + NH * CA + NH * C + 2


def _build():
    nc = bacc.Bacc(None, target_bir_lowering=False)
    qa = nc.declare_dram_parameter("qa", [CA, W_QA], BF, isOutput=False)
    xt = nc.declare_dram_parameter("xt", [128, JT * CA], BF, isOutput=False)
    out = nc.declare_dram_parameter("out", [C, SEQ], FP, isOutput=True)

    with tile.TileContext(nc) as tc:
        with (
            tc.tile_pool(name="consts", bufs=1) as consts,
            tc.tile_pool(name="hpool", bufs=4) as hpool,
            tc.tile_pool(name="sc", bufs=2, space="PSUM") as sc_psum,
            tc.tile_pool(name="pm", bufs=2, space="PSUM") as pm_psum,
        ):
            qa_sb = consts.tile([CA, W_QA], BF)
            # weights chunk first (gates R), then the two xa halves, each on
            # its own DMA queue
            nc.sync.dma_start(out=qa_sb[:, SEQ:], in_=qa[:, SEQ:])
            for ic in range(IC):
                nc.sync.dma_start(
                    out=qa_sb[:, ic * 512:(ic + 1) * 512],
                    in_=qa[:, ic * 512:(ic + 1) * 512],
                )
            xtb_sb = consts.tile([128, JT * CA], BF)
            nc.sync.dma_start(out=xtb_sb[:], in_=xt[:, :])
            xa_sb = qa_sb[:, 0:SEQ]
            bo_col = qa_sb[0:C, W_QA - 2:W_QA].bitcast(FP)

            def gt_view(h):
                return qa_sb[:, SEQ + h * CA: SEQ + (h + 1) * CA]

            def ff_view(h):
                return qa_sb[:, SEQ + NH * CA + h * C: SEQ + NH * CA + (h + 1) * C]

            def xt_view(jt):
                return xtb_sb[:, jt * CA:(jt + 1) * CA]

            out_sb = consts.tile([C, SEQ], FP)
            o2acc = consts.tile([C, SEQ], FP)

            # ---- HAM warm-up: dummy matmuls on a zeroed scratch tile keep
            # the PE busy during the input DMA so real matmuls start warm
            warm = consts.tile([128, 512], BF)
            nc.gpsimd.memset(warm[:], 0.0)
            wps = sc_psum.tile([128, SEQ], FP, tag="sc", name="warm")
            for w in range(N_WARM):
                nc.tensor.matmul(
                    wps[:, 0:512],
                    lhsT=warm[:, 0:128],
                    rhs=warm[:, 0:512],
                    start=True, stop=True,
                )

            def emit_R_ic(h, ic, state):
                if ic == 0:
                    state = (
                        hpool.tile([CA, SEQ], BF, tag="R", name=f"R_{h}"),
                        pm_psum.tile([CA, SEQ], FP, tag="pm", name=f"rp_{h}"),
                    )
                R_sb, rps = state
                nc.tensor.matmul(
                    rps[:, ic * 512:(ic + 1) * 512],
                    lhsT=gt_view(h),
                    rhs=xa_sb[:, ic * 512:(ic + 1) * 512],
                    start=True, stop=True,
                )
                nc.vector.tensor_copy(
                    out=R_sb[:, ic * 512:(ic + 1) * 512],
                    in_=rps[:, ic * 512:(ic + 1) * 512],
                )
                return state

            def emit_M2_mm(mps, xtr, es, jt, ic):
                nc.tensor.matmul(
                    mps[:, ic * 512:(ic + 1) * 512],
                    lhsT=xtr[:, jt, :],
                    rhs=es[:, jt, ic * 512:(ic + 1) * 512],
                    start=(jt == 0), stop=(jt == JT - 1),
                )

            # per-head state: (es, xtr, mps) created lazily
            R_cur = emit_R_ic(0, 0, None)
            R_cur = emit_R_ic(0, 1, R_cur)[0]
            R_nxt = None
            R_state = None
            prev = None     # (h, m2_sbuf, o2p) tail of the previous head
            cur = None      # (es, xtr, mps) of the current head

            def sl_ic(ic):
                return slice(ic * 512, (ic + 1) * 512)

            for h in range(NH):
                R_sb = R_cur
                last = h == NH - 1
                es = hpool.tile([128, JT, SEQ], BF, tag="es", name=f"es_{h}")
                xtr = hpool.tile([128, JT, CA], BF, tag="xtr", name=f"xtr_{h}")
                den = hpool.tile([128, JT], FP, tag="den", name=f"den_{h}")
                rec = hpool.tile([128, JT], FP, tag="rec", name=f"rec_{h}")
                pcur = cur
                cur = None

                for jt in range(JT):
                    pst = sc_psum.tile([128, SEQ], FP, tag="sc", name=f"sc_{h}_{jt}")
                    for ic in range(IC):
                        nc.tensor.matmul(
                            pst[:, sl_ic(ic)],
                            lhsT=xa_sb[:, jt * 128:(jt + 1) * 128],
                            rhs=R_sb[:, sl_ic(ic)],
                            start=True, stop=True,
                        )
                    nc.scalar.activation(
                        out=es[:, jt, :],
                        in_=pst[:],
                        func=AF.Exp,
                        scale=SCALE,
                        accum_out=den[:, jt:jt + 1],
                    )
                    nc.vector.reciprocal(out=rec[:, jt:jt + 1], in_=den[:, jt:jt + 1])
                    nc.vector.tensor_scalar_mul(
                        xtr[:, jt, :], xt_view(jt), rec[:, jt:jt + 1],
                    )

                    # ---- own M2 at lag-1 (allocate mps at jt==1)
                    if jt >= 1:
                        if jt == 1:
                            mps = pm_psum.tile([CA, SEQ], FP, tag="pm", name=f"mp_{h}")
                            cur = (es, xtr, mps)
                        for ic in range(IC):
                            emit_M2_mm(cur[2], xtr, es, jt - 1, ic)

                    # ---- tail of the previous head, one chunk per step
                    if pcur is not None:
                        pes, pxtr, pmps = pcur
                        ph = h - 1
                        if jt == 0:
                            # finish M2 (j-tile 7) and convert ic0
                            for ic in range(IC):
                                emit_M2_mm(pmps, pxtr, pes, JT - 1, ic)
                            pm2 = hpool.tile([CA, SEQ], BF, tag="m2", name=f"m2_{ph}")
                            nc.vector.tensor_copy(out=pm2[:, sl_ic(0)], in_=pmps[:, sl_ic(0)])
                        if jt == 1:
                            nc.vector.tensor_copy(out=pm2[:, sl_ic(1)], in_=pmps[:, sl_ic(1)])
                            o2p = pm_psum.tile([CA, SEQ], FP, tag="pm", name=f"o2_{ph}")
                            nc.tensor.matmul(
                                o2p[:C, sl_ic(0)], lhsT=ff_view(ph), rhs=pm2[:, sl_ic(0)],
                                start=True, stop=True,
                            )
                        if jt == 2:
                            nc.tensor.matmul(
                                o2p[:C, sl_ic(1)], lhsT=ff_view(ph), rhs=pm2[:, sl_ic(1)],
                                start=True, stop=True,
                            )
                            nc.vector.tensor_add(
                                out=o2acc[:, sl_ic(0)], in0=o2acc[:, sl_ic(0)],
                                in1=o2p[:C, sl_ic(0)],
                            )
                        if jt == 3:
                            nc.vector.tensor_add(
                                out=o2acc[:, sl_ic(1)], in0=o2acc[:, sl_ic(1)],
                                in1=o2p[:C, sl_ic(1)],
                            )
                            pcur = None

                    # ---- R for the next head
                    if jt == 4 and h + 1 < NH:
                        R_state = emit_R_ic(h + 1, 0, None)
                    if jt == 5 and h + 1 < NH:
                        R_nxt = emit_R_ic(h + 1, 1, R_state)[0]

                    # ---- residual+bias accumulator init (head 0 only)
                    if h == 0 and jt in (5, 6):
                        ic = jt - 5
                        nc.vector.tensor_scalar_add(
                            o2acc[:, sl_ic(ic)], qa_sb[0:C, sl_ic(ic)], bo_col,
                        )

                R_cur = R_nxt

            # ---- drain the last head: M2 tail, conv, out2, final add + DMA,
            # fully per-i-chunk so DVE/PE/DMA overlap
            es, xtr, mps = cur
            for ic in range(IC):
                emit_M2_mm(mps, xtr, es, JT - 1, ic)
            pm2 = hpool.tile([CA, SEQ], BF, tag="m2", name="m2_last")
            o2p = pm_psum.tile([CA, SEQ], FP, tag="pm", name="o2_last")
            for ic in range(IC):
                sl = sl_ic(ic)
                nc.vector.tensor_copy(out=pm2[:, sl], in_=mps[:, sl])
                nc.tensor.matmul(
                    o2p[:C, sl],
                    lhsT=ff_view(NH - 1),
                    rhs=pm2[:, sl],
                    start=True, stop=True,
                )
                nc.vector.tensor_add(
                    out=out_sb[:, sl], in0=o2acc[:, sl], in1=o2p[:C, sl],
                )
                nc.sync.dma_start(out=out[:, sl], in_=out_sb[:, sl])

    nc.compile()
    return nc


_CACHE: dict = {}


def _get_nc():
    if "nc" not in _CACHE:
        _CACHE["nc"] = _build()
    return _CACHE["nc"]


def _prep_in_maps(x, W_proj, b_proj, W_out, b_out):
    bf = ml_dtypes.bfloat16
    x = np.ascontiguousarray(np.asarray(x, dtype=np.float32))
    W_proj = np.asarray(W_proj, dtype=np.float32)
    b_proj = np.asarray(b_proj, dtype=np.float32)
    W_out = np.asarray(W_out, dtype=np.float32)
    b_out = np.asarray(b_out, dtype=np.float32)

    x2 = x.reshape(N_CORES, C, SEQ)

    # augmented per-head projection blocks [65, 640]
    Wa = np.concatenate([W_proj, b_proj[None, :]], axis=0)  # [65, 7680]
    gt = np.empty((CA, NH, CA), dtype=np.float32)
    ffm = np.empty((CA, NH, C), dtype=np.float32)
    for h in range(NH):
        q0 = h * 3 * D
        Wq = Wa[:, q0:q0 + D]            # [65, 640]
        Wk = Wa[:, q0 + D:q0 + 2 * D]
        Wv = Wa[:, q0 + 2 * D:q0 + 3 * D]
        G = Wk @ Wq.T                    # [65, 65]; scoresT = Xa^T G Xa
        gt[:, h, :] = G.T                # lhsT[c', c] = G[c, c']
        ffm[:, h, :] = Wv @ W_out[h * D:(h + 1) * D, :]   # [65, 64]

    # blob 1: xa | gt | ff | b_out  on 65 partitions
    qa_all = np.empty((N_CORES, CA, W_QA), dtype=bf)
    qa_all[:, :C, :SEQ] = x2.astype(bf)
    qa_all[:, C, :SEQ] = np.float32(1.0)
    qa_all[:, :, SEQ:SEQ + NH * CA] = gt.reshape(CA, NH * CA).astype(bf)[None]
    qa_all[:, :, SEQ + NH * CA:SEQ + NH * CA + NH * C] = (
        ffm.reshape(CA, NH * C).astype(bf)[None]
    )
    qa_all[:, :C, W_QA - 2:] = (
        b_out.astype(np.float32).view(bf).reshape(C, 2)[None]
    )
    qa_all[:, C, W_QA - 2:] = np.float32(0.0)

    # blob 2: XaT [p, jt*65+c'] = x[c', jt*128+p], ones at c'=64
    xt_all = np.empty((N_CORES, 128, JT, CA), dtype=bf)
    xtt = x2.transpose(0, 2, 1).reshape(N_CORES, JT, 128, C)  # [b, jt, p, c]
    xt_all[:, :, :, :C] = xtt.transpose(0, 2, 1, 3).astype(bf)
    xt_all[:, :, :, C] = np.float32(1.0)
    xt_all = xt_all.reshape(N_CORES, 128, JT * CA)

    return [
        {
            "qa": np.ascontiguousarray(qa_all[i]),
            "xt": np.ascontiguousarray(xt_all[i]),
        }
        for i in range(N_CORES)
    ]


def run(x, t, W_proj, b_proj, W_out, b_out, trace=False, **trace_kwargs):
    in_maps = _prep_in_maps(x, W_proj, b_proj, W_out, b_out)
    res = run_bass_kernel_spmd(
        _get_nc(), in_maps, core_ids=list(range(N_CORES)),
        trace=trace, **trace_kwargs,
    )
    out = np.stack([res.results[i]["out"] for i in range(N_CORES)])
    return out.reshape(N_CORES, C, 32, 32), res


def kernel(x, t=None, W_proj=None, b_proj=None, W_out=None, b_out=None):
    out, _ = run(x, t, W_proj, b_proj, W_out, b_out, trace=False)
    return out
